# revision 2
# baseline (speedup 1.0000x reference)
"""Trainium2 Bass kernel for ClassicalSelfAttention.

  out = softmax((x @ Wq) @ (x @ Wk)^T / sqrt(D)) @ x      x: [8192, 1024] f32

Sharding (8 NeuronCores): rows of x are sharded across cores; each core
projects its own row-shard to Q^T and K^T, the K^T shards are AllGathered
across cores (SDMA, overlaps compute), and each core runs a streaming
attention loop over 16 key-blocks of 512 keys: scores matmul -> fused
exp+rowsum on ScalarE -> PV matmul accumulated in SBUF. The softmax
division is folded into the final output scale; the 1/sqrt(1024) = 2^-5
logit scale is applied inside the exp activation (exact in fp32).

The scores matmul runs in fp8 e4m3 with the DoubleRow perf mode (two
128-deep contraction tiles per instruction at 0.5 cycles/row). To keep
the accuracy inside the correctness gate, Q^T is kept as an fp8 hi+lo
pair (qt_lo = Q^T - fp8(Q^T), re-quantized to fp8 -- the residual lands
in e4m3's subnormal range where it keeps ~3 more mantissa bits), while
K^T is a single fp8 tensor; the scores accumulation sums K^T.T @ Q_hi
and K^T.T @ Q_lo into the same PSUM bank. The PV matmul stays bf16 with
fp32 PSUM accumulation.

The scores matmul keeps K^T stationary and Q^T moving, so PSUM holds
scores TRANSPOSED ([key, query]); exp of that is P^T directly -- which
is exactly the layout the PV matmul needs as its stationary operand --
so no PE transposes are required at all. The softmax row-sums (a
partition-dim reduction in this layout) are computed by a ones-vector
matmul and fixed up into per-partition scalars at the end.

To hide the AllGather latency each core processes its OWN key blocks
first straight out of SBUF (plus its own V rows from a per-core x_shard
input); the remaining 14 key blocks are fetched in rank-rotated order
(rank + j) % 8 via partition-id-based dynamic DMA offsets, so no core
waits on the gather before doing useful work. Softmax over key blocks
is order-invariant, so the rotation is free.
"""

import sys

import numpy as np

try:
    import concourse.bass as bass  # noqa: F401
except ImportError:  # pragma: no cover
    sys.path.insert(0, "/opt/trn_rl_repo")

import concourse.bacc as bacc
import concourse.mybir as mybir
import concourse.tile as tile
from concourse.masks import make_identity
from concourse import bass_utils
from concourse.bass import ds

N_TOKENS = 8192
EMBED = 1024
NCORES = 8
M = N_TOKENS // NCORES  # rows per core (1024)
P = 128  # partitions
DC = EMBED // P  # contraction chunks (8)
NB = 512  # key-block width
NNB = N_TOKENS // NB  # key blocks (16)
MB = M // P  # query row-blocks per core (8)
VC = NB // P  # value chunks per key block (4)
HPR = M // NB  # key-block halves per rank (2)
FP32 = mybir.dt.float32
BF16 = mybir.dt.bfloat16
FP8 = mybir.dt.float8e4
EXP = mybir.ActivationFunctionType.Exp
ADD = mybir.AluOpType.add
SUB = mybir.AluOpType.subtract
DROW = mybir.MatmulPerfMode.DoubleRow
SCALE = 1.0 / 32.0  # 1/sqrt(EMBED), applied inside the exp activation


def _build():
    nc = bacc.Bacc(
        "TRN2", target_bir_lowering=False, debug=False, num_devices=NCORES
    )
    xt_shard = nc.dram_tensor("xt_shard", [EMBED, M], BF16, kind="ExternalInput").ap()
    x_shard = nc.dram_tensor("x_shard", [M, EMBED], BF16, kind="ExternalInput").ap()
    x_full = nc.dram_tensor(
        "x_full", [N_TOKENS, EMBED], BF16, kind="ExternalInput"
    ).ap()
    wq_d = nc.dram_tensor("wq", [EMBED, EMBED], BF16, kind="ExternalInput").ap()
    wk_d = nc.dram_tensor("wk", [EMBED, EMBED], BF16, kind="ExternalInput").ap()
    out_d = nc.dram_tensor("out", [M, EMBED], FP32, kind="ExternalOutput").ap()

    wq_r = wq_d.rearrange("(a p) d -> a p d", p=P)  # [DC, P, EMBED]
    wk_r = wk_d.rearrange("(a p) d -> a p d", p=P)
    xt_r = xt_shard.rearrange("(a p) m -> a p m", p=P)  # [DC, P, M]
    xs_r = x_shard.rearrange("(t p) d -> t p d", p=P)  # [M//P, P, EMBED]
    xv_r = x_full.rearrange("(t p) d -> t p d", p=P)  # [64, P, EMBED]
    out_r = out_d.rearrange("(t p) d -> t p d", p=P)  # [MB, P, EMBED]

    with tile.TileContext(nc) as tc:
        with (
            tc.tile_pool(name="persist", bufs=1) as pers,
            tc.tile_pool(name="persist_dram", bufs=1, space="DRAM") as pdram,
        ):
            ones_sb = pers.tile([P, P], BF16)
            nc.vector.memset(ones_sb[:], 1.0)
            ident = pers.tile([P, P], FP32)
            make_identity(nc, ident[:])
            # Q^T resident for the whole kernel as an fp8 hi+lo pair:
            # qt_*[p, b*M + m] = Qt[b*128+p, m]
            qt_hi = pers.tile([P, DC * M], FP8)
            qt_lo = pers.tile([P, DC * M], FP8)
            # own K^T shard, kept resident: ktsb[p, b*M + n] = Kt_own[b*128+p, n]
            ktsb = pers.tile([P, DC * M], FP8)
            # fp32 PV accumulator per query block: [p, mb*EMBED + dv]
            out_acc = pers.tile([P, MB * EMBED], FP32)
            # softmax denominators, replicated across partitions: [p, m]
            sums_acc = pers.tile([P, M], FP32)
            # K^T shard (AllGather input) and gathered K^T of all cores
            ktd = pdram.tile([DC, P, M], FP8)
            gkt = pdram.tile([NCORES * DC, P, M], FP8, addr_space="Shared")

            rank = nc.gpsimd.partition_id()

            # ---- Phase A: project Q^T (own rows) and K^T shard, AllGather K^T
            with (
                tc.tile_pool(name="proj", bufs=1) as proj,
                tc.tile_pool(name="proj_ps", bufs=4, space="PSUM") as proj_ps,
            ):
                wq_sb = proj.tile([P, DC * EMBED], BF16)
                wk_sb = proj.tile([P, DC * EMBED], BF16)
                xt_sb = proj.tile([P, DC * M], BF16)
                for a in range(DC):
                    nc.sync.dma_start(
                        out=wk_sb[:, a * EMBED : (a + 1) * EMBED], in_=wk_r[a]
                    )
                    nc.sync.dma_start(
                        out=xt_sb[:, a * M : (a + 1) * M], in_=xt_r[a]
                    )
                    nc.sync.dma_start(
                        out=wq_sb[:, a * EMBED : (a + 1) * EMBED], in_=wq_r[a]
                    )
                # K^T first so its AllGather overlaps the Q^T projection.
                for w_sb, dst in ((wk_sb, ktsb), (wq_sb, qt_hi)):
                    for b in range(DC):  # output dim chunk
                        for j in range(M // NB):  # row half
                            ps = proj_ps.tile([P, NB], FP32, tag="proj_ps")
                            for a in range(DC):  # contraction chunk
                                nc.tensor.matmul(
                                    ps[:],
                                    lhsT=w_sb[:, a * EMBED + b * P : a * EMBED + (b + 1) * P],
                                    rhs=xt_sb[:, a * M + j * NB : a * M + (j + 1) * NB],
                                    start=(a == 0),
                                    stop=(a == DC - 1),
                                )
                            sl = slice(b * M + j * NB, b * M + (j + 1) * NB)
                            if dst is ktsb:
                                nc.vector.tensor_copy(out=ktsb[:, sl], in_=ps[:])
                            else:
                                # fp8 hi + (residual -> fp8) lo pair
                                nc.vector.tensor_copy(out=qt_hi[:, sl], in_=ps[:])
                                nc.vector.tensor_tensor(
                                    out=qt_lo[:, sl],
                                    in0=ps[:],
                                    in1=qt_hi[:, sl],
                                    op=SUB,
                                )
                    if dst is ktsb:
                        for b in range(DC):
                            nc.sync.dma_start(
                                out=ktd[b], in_=ktsb[:, b * M : (b + 1) * M]
                            )
                        nc.gpsimd.collective_compute(
                            "AllGather",
                            mybir.AluOpType.bypass,
                            replica_groups=[list(range(NCORES))],
                            ins=[ktd.opt()],
                            outs=[gkt.opt()],
                        )

            qh_v = qt_hi[:].rearrange("p (b m) -> p b m", b=DC)  # [P, DC, M]
            ql_v = qt_lo[:].rearrange("p (b m) -> p b m", b=DC)

            # ---- Phase B: streaming attention over key blocks, own rank first
            with (
                tc.tile_pool(name="kv", bufs=2) as kvp,
                tc.tile_pool(name="pb", bufs=3) as pbp,
                tc.tile_pool(name="ps_s", bufs=3, space="PSUM") as ps_sp,
                tc.tile_pool(name="ps_u", bufs=2, space="PSUM") as ps_up,
                tc.tile_pool(name="ps_o", bufs=2, space="PSUM") as ps_op,
            ):
                for nb in range(NNB):  # local processing order
                    j, half = nb // HPR, nb % HPR  # j = rank offset
                    vtile = kvp.tile([P, VC * EMBED], BF16, tag="vtile")
                    if j == 0:
                        # own keys: K^T already in SBUF, V rows from x_shard
                        for c in range(VC):
                            nc.sync.dma_start(
                                out=vtile[:, c * EMBED : (c + 1) * EMBED],
                                in_=xs_r[half * VC + c],
                            )
                        kt_v = ktsb[:].rearrange("p (b m) -> p b m", b=DC)
                        k_off = half * NB
                    else:
                        src = (rank + j) % NCORES
                        for c in range(VC):
                            nc.gpsimd.dma_start(
                                out=vtile[:, c * EMBED : (c + 1) * EMBED],
                                in_=xv_r[
                                    ds(src * (M // P) + half * VC + c, 1)
                                ].squeeze(0),
                            )
                        ktile = kvp.tile([P, DC * NB], FP8, tag="ktile")
                        for b in range(DC):
                            nc.gpsimd.dma_start(
                                out=ktile[:, b * NB : (b + 1) * NB],
                                in_=gkt[
                                    ds(src * DC + b, 1),
                                    :,
                                    half * NB : (half + 1) * NB,
                                ].squeeze(0),
                            )
                        kt_v = ktile[:].rearrange("p (b n) -> p b n", b=DC)
                        k_off = 0

                    pt_sb = pbp.tile([P, VC * M], BF16, tag="pt_sb")
                    for h in range(M // NB):  # query column half
                        for c in range(VC):  # key chunk within block
                            ps_s = ps_sp.tile([P, NB], FP32, tag="ps_s")
                            # fp8 DoubleRow: 2 contraction tiles / instruction,
                            # 4 hi-instructions then 4 lo-instructions.
                            for qi, q_v in enumerate((qh_v, ql_v)):
                                for bb in range(DC // 2):
                                    nc.tensor.matmul(
                                        ps_s[:],
                                        lhsT=kt_v[
                                            :,
                                            2 * bb : 2 * bb + 2,
                                            k_off + c * P : k_off + (c + 1) * P,
                                        ],
                                        rhs=q_v[
                                            :,
                                            2 * bb : 2 * bb + 2,
                                            h * NB : (h + 1) * NB,
                                        ],
                                        start=(qi == 0 and bb == 0),
                                        stop=(qi == 1 and bb == DC // 2 - 1),
                                        perf_mode=DROW,
                                    )
                            nc.scalar.activation(
                                out=pt_sb[:, c * M + h * NB : c * M + (h + 1) * NB],
                                in_=ps_s[:],
                                func=EXP,
                                scale=SCALE,
                            )
                    # partition-dim softmax sums via ones-vector matmul
                    for h in range(M // NB):
                        ps_sum = ps_up.tile([P, NB], FP32, tag="ps_sum")
                        for c in range(VC):
                            nc.tensor.matmul(
                                ps_sum[:],
                                lhsT=ones_sb[:],
                                rhs=pt_sb[:, c * M + h * NB : c * M + (h + 1) * NB],
                                start=(c == 0),
                                stop=(c == VC - 1),
                            )
                        dsts = sums_acc[:, h * NB : (h + 1) * NB]
                        if nb == 0:
                            nc.vector.tensor_copy(out=dsts, in_=ps_sum[:])
                        else:
                            nc.vector.tensor_tensor(
                                out=dsts, in0=dsts, in1=ps_sum[:], op=ADD
                            )
                    for mb in range(MB):
                        for h in range(EMBED // NB):
                            ps_o = ps_op.tile([P, NB], FP32, tag="ps_o")
                            for t in range(VC):
                                nc.tensor.matmul(
                                    ps_o[:],
                                    lhsT=pt_sb[:, t * M + mb * P : t * M + (mb + 1) * P],
                                    rhs=vtile[:, t * EMBED + h * NB : t * EMBED + (h + 1) * NB],
                                    start=(t == 0),
                                    stop=(t == VC - 1),
                                )
                            dst = out_acc[:, mb * EMBED + h * NB : mb * EMBED + (h + 1) * NB]
                            if nb == 0:
                                nc.vector.tensor_copy(out=dst, in_=ps_o[:])
                            else:
                                nc.vector.tensor_tensor(
                                    out=dst, in0=dst, in1=ps_o[:], op=ADD
                                )

            # ---- Phase C: divide by softmax sum, write out
            with (
                tc.tile_pool(name="fin", bufs=2) as fin,
                tc.tile_pool(name="fin_ps", bufs=2, space="PSUM") as fin_ps,
            ):
                scol = fin.tile([P, MB], FP32)
                for mb in range(MB):
                    ps_f = fin_ps.tile([P, P], FP32, tag="ps_f")
                    nc.tensor.transpose(
                        out=ps_f[:],
                        in_=sums_acc[:, mb * P : (mb + 1) * P],
                        identity=ident[:],
                    )
                    nc.vector.tensor_copy(
                        out=scol[:, mb : mb + 1], in_=ps_f[:, 0:1]
                    )
                rtot = fin.tile([P, MB], FP32)
                nc.vector.reciprocal(out=rtot[:], in_=scol[:])
                for mb in range(MB):
                    outf = fin.tile([P, EMBED], FP32, tag="outf")
                    nc.vector.tensor_scalar_mul(
                        outf[:],
                        out_acc[:, mb * EMBED : (mb + 1) * EMBED],
                        rtot[:, mb : mb + 1],
                    )
                    nc.sync.dma_start(out=out_r[mb], in_=outf[:])

    nc.compile()
    return nc


_NC = None


def _get_nc():
    global _NC
    if _NC is None:
        _NC = _build()
    return _NC


def _run(x, rotation_params, entangle_params, **spmd_kwargs):
    x = np.ascontiguousarray(np.asarray(x, dtype=np.float32))
    wq = np.asarray(rotation_params, dtype=np.float32).reshape(EMBED, EMBED)
    wk = np.asarray(entangle_params, dtype=np.float32).reshape(EMBED, EMBED)
    import ml_dtypes

    bf = ml_dtypes.bfloat16
    x_bf = x.astype(bf)
    xt_bf = np.ascontiguousarray(x.T.astype(bf))
    wq_bf = wq.astype(bf)
    wk_bf = wk.astype(bf)
    in_maps = [
        {
            "xt_shard": np.ascontiguousarray(xt_bf[:, i * M : (i + 1) * M]),
            "x_shard": np.ascontiguousarray(x_bf[i * M : (i + 1) * M]),
            "x_full": x_bf,
            "wq": wq_bf,
            "wk": wk_bf,
        }
        for i in range(NCORES)
    ]
    res = bass_utils.run_bass_kernel_spmd(
        _get_nc(), in_maps, core_ids=list(range(NCORES)), **spmd_kwargs
    )
    out = np.concatenate([res.results[i]["out"] for i in range(NCORES)], axis=0)
    return out, res


def kernel(x, rotation_params, entangle_params):
    out, _ = _run(x, rotation_params, entangle_params)
    return out


# revision 3
# speedup vs baseline: 1.2497x; 1.2497x over previous
"""Trainium2 Bass kernel for ClassicalSelfAttention.

  out = softmax((x @ Wq) @ (x @ Wk)^T / sqrt(D)) @ x      x: [8192, 1024] f32

Key algebraic restructuring: scores = (x Wq)(x Wk)^T = x (Wq Wk^T) x^T, so
the two weight matrices are folded offline (host-side, weight-only, input
independent) into Z = Wq Wk^T.  On device each core projects only its own
row-shard once (qt = x_own @ 8Z, the 8x pre-scale keeps the fp8 residual
in e4m3's normal range) and computes its scores row-block directly against
x^T -- which every core already holds as a kernel input for the PV stage.
This removes the K projection AND the K^T AllGather entirely: no
collective, no inter-core dependency, all 16 key blocks stream uniformly
from DRAM.

Sharding (8 NeuronCores): rows of x are sharded across cores; each core
runs a streaming attention loop over 16 key-blocks of 512 keys: fp8
scores matmul -> fused exp (with the 1/(32*8) logit scale) on ScalarE ->
fp16 PV matmul accumulated in SBUF fp32.  The softmax division is folded
into the final output scale.

The scores matmul runs in fp8 e4m3 with the DoubleRow perf mode (two
128-deep contraction tiles per instruction -> 2x MACs per PE cycle).  To
stay inside the correctness gate, q̃ is kept as an fp8 hi+lo pair
(lo = q̃ - fp8(q̃), re-quantized to fp8) and the scores accumulate
x8^T @ q_hi over all 1024 contraction dims plus x8^T @ q_lo over the
first 512 (the residual correction is half-depth: full depth costs as
much as bf16, half keeps ~85% of the accuracy win).  The K-side operand
is host-quantized x^T in fp8.  The PV matmul runs fp16 with fp32 PSUM.

The scores matmul keeps x^T stationary and q̃^T moving, so PSUM holds
scores TRANSPOSED ([key, query]); exp of that is P^T directly -- exactly
the layout the PV matmul needs as its stationary operand -- so no PE
transposes are needed.  The softmax row-sums (a partition-dim reduction
in this layout) are computed by a ones-vector matmul and fixed up into
per-partition scalars at the end.
"""

import sys

import numpy as np

try:
    import concourse.bass as bass  # noqa: F401
except ImportError:  # pragma: no cover
    sys.path.insert(0, "/opt/trn_rl_repo")

import concourse.bacc as bacc
import concourse.mybir as mybir
import concourse.tile as tile
from concourse.masks import make_identity
from concourse import bass_utils

N_TOKENS = 8192
EMBED = 1024
NCORES = 8
M = N_TOKENS // NCORES  # rows per core (1024)
P = 128  # partitions
DC = EMBED // P  # contraction chunks (8)
NB = 512  # key-block width
NNB = N_TOKENS // NB  # key blocks (16)
MB = M // P  # query row-blocks per core (8)
VC = NB // P  # value chunks per key block (4)
LOC = DC // 2  # contraction chunks covered by the lo-residual pass (4)
FP32 = mybir.dt.float32
BF16 = mybir.dt.bfloat16
FP16 = mybir.dt.float16
FP8 = mybir.dt.float8e4
EXP = mybir.ActivationFunctionType.Exp
ADD = mybir.AluOpType.add
SUB = mybir.AluOpType.subtract
DROW = mybir.MatmulPerfMode.DoubleRow
# logits scale: 1/sqrt(EMBED) softmax scale x 1/8 undoing the 8*Z prescale
SCALE = 1.0 / 256.0


def _build():
    nc = bacc.Bacc(
        "TRN2", target_bir_lowering=False, debug=False, num_devices=NCORES
    )
    xt_shard = nc.dram_tensor("xt_shard", [EMBED, M], BF16, kind="ExternalInput").ap()
    xt8_full = nc.dram_tensor(
        "xt8_full", [EMBED, N_TOKENS], FP8, kind="ExternalInput"
    ).ap()
    x16_full = nc.dram_tensor(
        "x16_full", [N_TOKENS, EMBED], FP16, kind="ExternalInput"
    ).ap()
    z_d = nc.dram_tensor("z", [EMBED, EMBED], BF16, kind="ExternalInput").ap()
    out_d = nc.dram_tensor("out", [M, EMBED], BF16, kind="ExternalOutput").ap()

    z_r = z_d.rearrange("(a p) d -> a p d", p=P)  # [DC, P, EMBED]
    xt_r = xt_shard.rearrange("(a p) m -> a p m", p=P)  # [DC, P, M]
    xt8_r = xt8_full.rearrange("(a p) n -> a p n", p=P)  # [DC, P, N]
    xv_r = x16_full.rearrange("(t p) d -> t p d", p=P)  # [64, P, EMBED]
    out_r = out_d.rearrange("(t p) d -> t p d", p=P)  # [MB, P, EMBED]

    with tile.TileContext(nc) as tc:
        with tc.tile_pool(name="persist", bufs=1) as pers:
            ones_sb = pers.tile([P, P], FP16)
            nc.vector.memset(ones_sb[:], 1.0)
            ident = pers.tile([P, P], FP32)
            make_identity(nc, ident[:])
            # q~^T resident as an fp8 hi+lo pair: qt_*[p, b*M + m]
            qt_hi = pers.tile([P, DC * M], FP8)
            qt_lo = pers.tile([P, DC * M], FP8)
            # fp32 PV accumulator per query block: [p, mb*EMBED + dv]
            out_acc = pers.tile([P, MB * EMBED], FP32)
            # softmax denominators, replicated across partitions: [p, m]
            sums_acc = pers.tile([P, M], FP32)

            # ---- Phase A: project q~^T = (8Z)^T @ x_own^T
            with (
                tc.tile_pool(name="proj", bufs=1) as proj,
                tc.tile_pool(name="proj_ps", bufs=4, space="PSUM") as proj_ps,
            ):
                z_sb = proj.tile([P, DC * EMBED], BF16)
                xt_sb = proj.tile([P, DC * M], BF16)
                for a in range(DC):
                    nc.sync.dma_start(
                        out=z_sb[:, a * EMBED : (a + 1) * EMBED], in_=z_r[a]
                    )
                    nc.sync.dma_start(
                        out=xt_sb[:, a * M : (a + 1) * M], in_=xt_r[a]
                    )
                for b in range(DC):  # output dim chunk
                    for j in range(M // NB):  # row half
                        ps = proj_ps.tile([P, NB], FP32, tag="proj_ps")
                        for a in range(DC):  # contraction chunk
                            nc.tensor.matmul(
                                ps[:],
                                lhsT=z_sb[:, a * EMBED + b * P : a * EMBED + (b + 1) * P],
                                rhs=xt_sb[:, a * M + j * NB : a * M + (j + 1) * NB],
                                start=(a == 0),
                                stop=(a == DC - 1),
                            )
                        sl = slice(b * M + j * NB, b * M + (j + 1) * NB)
                        nc.vector.tensor_copy(out=qt_hi[:, sl], in_=ps[:])
                        nc.vector.tensor_tensor(
                            out=qt_lo[:, sl], in0=ps[:], in1=qt_hi[:, sl], op=SUB
                        )

            qh_v = qt_hi[:].rearrange("p (b m) -> p b m", b=DC)  # [P, DC, M]
            ql_v = qt_lo[:].rearrange("p (b m) -> p b m", b=DC)

            # ---- Phase B: streaming attention over the 16 key blocks
            with (
                tc.tile_pool(name="kv", bufs=2) as kvp,
                tc.tile_pool(name="pb", bufs=3) as pbp,
                tc.tile_pool(name="ps_s", bufs=3, space="PSUM") as ps_sp,
                tc.tile_pool(name="ps_u", bufs=2, space="PSUM") as ps_up,
                tc.tile_pool(name="ps_o", bufs=2, space="PSUM") as ps_op,
            ):
                for nb in range(NNB):
                    vtile = kvp.tile([P, VC * EMBED], FP16, tag="vtile")
                    for c in range(VC):
                        nc.sync.dma_start(
                            out=vtile[:, c * EMBED : (c + 1) * EMBED],
                            in_=xv_r[nb * VC + c],
                        )
                    ktile = kvp.tile([P, DC * NB], FP8, tag="ktile")
                    for b in range(DC):
                        nc.sync.dma_start(
                            out=ktile[:, b * NB : (b + 1) * NB],
                            in_=xt8_r[b, :, nb * NB : (nb + 1) * NB],
                        )
                    kt_v = ktile[:].rearrange("p (b n) -> p b n", b=DC)

                    pt_sb = pbp.tile([P, VC * M], FP16, tag="pt_sb")
                    for h in range(M // NB):  # query column half
                        for c in range(VC):  # key chunk within block
                            ps_s = ps_sp.tile([P, NB], FP32, tag="ps_s")
                            # fp8 DoubleRow: 2 contraction tiles/instruction;
                            # hi over all DC tiles, lo over the first LOC.
                            for qi, q_v, nbb in ((0, qh_v, DC // 2), (1, ql_v, LOC // 2)):
                                for bb in range(nbb):
                                    nc.tensor.matmul(
                                        ps_s[:],
                                        lhsT=kt_v[
                                            :,
                                            2 * bb : 2 * bb + 2,
                                            c * P : (c + 1) * P,
                                        ],
                                        rhs=q_v[
                                            :,
                                            2 * bb : 2 * bb + 2,
                                            h * NB : (h + 1) * NB,
                                        ],
                                        start=(qi == 0 and bb == 0),
                                        stop=(qi == 1 and bb == LOC // 2 - 1),
                                        perf_mode=DROW,
                                    )
                            nc.scalar.activation(
                                out=pt_sb[:, c * M + h * NB : c * M + (h + 1) * NB],
                                in_=ps_s[:],
                                func=EXP,
                                scale=SCALE,
                            )
                    # partition-dim softmax sums via ones-vector matmul
                    for h in range(M // NB):
                        ps_sum = ps_up.tile([P, NB], FP32, tag="ps_sum")
                        for c in range(VC):
                            nc.tensor.matmul(
                                ps_sum[:],
                                lhsT=ones_sb[:],
                                rhs=pt_sb[:, c * M + h * NB : c * M + (h + 1) * NB],
                                start=(c == 0),
                                stop=(c == VC - 1),
                            )
                        dsts = sums_acc[:, h * NB : (h + 1) * NB]
                        if nb == 0:
                            nc.vector.tensor_copy(out=dsts, in_=ps_sum[:])
                        else:
                            nc.vector.tensor_tensor(
                                out=dsts, in0=dsts, in1=ps_sum[:], op=ADD
                            )
                    for mb in range(MB):
                        for h in range(EMBED // NB):
                            ps_o = ps_op.tile([P, NB], FP32, tag="ps_o")
                            for t in range(VC):
                                nc.tensor.matmul(
                                    ps_o[:],
                                    lhsT=pt_sb[:, t * M + mb * P : t * M + (mb + 1) * P],
                                    rhs=vtile[:, t * EMBED + h * NB : t * EMBED + (h + 1) * NB],
                                    start=(t == 0),
                                    stop=(t == VC - 1),
                                )
                            dst = out_acc[:, mb * EMBED + h * NB : mb * EMBED + (h + 1) * NB]
                            if nb == 0:
                                nc.vector.tensor_copy(out=dst, in_=ps_o[:])
                            else:
                                nc.vector.tensor_tensor(
                                    out=dst, in0=dst, in1=ps_o[:], op=ADD
                                )

            # ---- Phase C: divide by softmax sum, write out
            with (
                tc.tile_pool(name="fin", bufs=2) as fin,
                tc.tile_pool(name="fin_ps", bufs=2, space="PSUM") as fin_ps,
            ):
                scol = fin.tile([P, MB], FP32)
                for mb in range(MB):
                    ps_f = fin_ps.tile([P, P], FP32, tag="ps_f")
                    nc.tensor.transpose(
                        out=ps_f[:],
                        in_=sums_acc[:, mb * P : (mb + 1) * P],
                        identity=ident[:],
                    )
                    nc.vector.tensor_copy(
                        out=scol[:, mb : mb + 1], in_=ps_f[:, 0:1]
                    )
                rtot = fin.tile([P, MB], FP32)
                nc.vector.reciprocal(out=rtot[:], in_=scol[:])
                for mb in range(MB):
                    outf = fin.tile([P, EMBED], BF16, tag="outf")
                    nc.vector.tensor_scalar_mul(
                        outf[:],
                        out_acc[:, mb * EMBED : (mb + 1) * EMBED],
                        rtot[:, mb : mb + 1],
                    )
                    nc.sync.dma_start(out=out_r[mb], in_=outf[:])

    nc.compile()
    return nc


_NC = None


def _get_nc():
    global _NC
    if _NC is None:
        _NC = _build()
    return _NC


def _run(x, rotation_params, entangle_params, **spmd_kwargs):
    x = np.ascontiguousarray(np.asarray(x, dtype=np.float32))
    wq = np.asarray(rotation_params, dtype=np.float32).reshape(EMBED, EMBED)
    wk = np.asarray(entangle_params, dtype=np.float32).reshape(EMBED, EMBED)
    import ml_dtypes

    # offline weight folding: Z = 8 * Wq Wk^T (the 8x keeps the device-side
    # fp8 residual of q~ = x @ 8Z in e4m3's normal range; undone in the exp)
    z8 = (8.0 * (wq @ wk.T)).astype(ml_dtypes.bfloat16)
    xt = np.ascontiguousarray(x.T)
    xt_bf = xt.astype(ml_dtypes.bfloat16)
    xt8 = xt.astype(ml_dtypes.float8_e4m3)
    x16 = x.astype(np.float16)
    in_maps = [
        {
            "xt_shard": np.ascontiguousarray(xt_bf[:, i * M : (i + 1) * M]),
            "xt8_full": xt8,
            "x16_full": x16,
            "z": z8,
        }
        for i in range(NCORES)
    ]
    res = bass_utils.run_bass_kernel_spmd(
        _get_nc(), in_maps, core_ids=list(range(NCORES)), **spmd_kwargs
    )
    out = np.concatenate(
        [res.results[i]["out"].astype(np.float32) for i in range(NCORES)], axis=0
    )
    return out, res


def kernel(x, rotation_params, entangle_params):
    out, _ = _run(x, rotation_params, entangle_params)
    return out


# revision 10
# speedup vs baseline: 1.5271x; 1.2220x over previous
"""Trainium2 Bass kernel for ClassicalSelfAttention.

  out = softmax((x @ Wq) @ (x @ Wk)^T / sqrt(D)) @ x      x: [8192, 1024] f32

Key algebraic restructuring: scores = (x Wq)(x Wk)^T = x (Wq Wk^T) x^T, so
the two weight matrices are folded offline (host-side, weight-only, input
independent) into Z = Wq Wk^T.  On device each core projects only its own
row-shard once (qt = x_own @ 8Z, the 8x pre-scale keeps the fp8 residual
in e4m3's normal range) and computes its scores row-block directly against
x^T -- which every core already holds as a kernel input for the PV stage.
This removes the K projection AND the K^T AllGather entirely: no
collective, no inter-core dependency, all 16 key blocks stream uniformly
from DRAM.

Sharding (8 NeuronCores): rows of x are sharded across cores; each core
runs a streaming attention loop over 16 key-blocks of 512 keys: fp8
scores matmul -> fused exp (with the 1/(32*8) logit scale) on ScalarE ->
fp16 PV matmul accumulated in SBUF fp32.  The softmax division is folded
into the final output scale.

The scores matmul runs in fp8 e4m3 with the DoubleRow perf mode (two
128-deep contraction tiles per instruction -> 2x MACs per PE cycle).  To
stay inside the correctness gate, q̃ is kept as an fp8 hi+lo pair
(lo = q̃ - fp8(q̃), re-quantized to fp8) and the scores accumulate
x8^T @ q_hi over all 1024 contraction dims plus a half-depth residual
x8^T @ q_lo over 512 dims (full depth would cost as much as bf16).
WHICH half the residual covers alternates per 128-key chunk: the two
coherent per-query error vectors from the uncovered halves then hit
disjoint key sets and add in quadrature instead of coherently, cutting
the residual error variance 2x at zero extra instructions.  The K-side
operand is host-quantized x^T in fp8.  The PV matmul runs fp16 with
fp32 PSUM.

The scores matmul keeps x^T stationary and q̃^T moving, so PSUM holds
scores TRANSPOSED ([key, query]); exp of that is P^T directly -- exactly
the layout the PV matmul needs as its stationary operand -- so no PE
transposes are needed.  The softmax row-sums (a partition-dim reduction
in this layout) are computed by a ones-vector matmul and fixed up into
per-partition scalars at the end.
"""

import sys

import numpy as np

try:
    import concourse.bass as bass  # noqa: F401
except ImportError:  # pragma: no cover
    sys.path.insert(0, "/opt/trn_rl_repo")

import concourse.bacc as bacc
import concourse.mybir as mybir
import concourse.tile as tile
from concourse.masks import make_identity
from concourse import bass_utils

N_TOKENS = 8192
EMBED = 1024
NCORES = 8
M = N_TOKENS // NCORES  # rows per core (1024)
P = 128  # partitions
DC = EMBED // P  # contraction chunks (8)
NB = 512  # key-block width
NNB = N_TOKENS // NB  # key blocks (16)
MB = M // P  # query row-blocks per core (8)
VC = NB // P  # value chunks per key block (4)
LOC = DC // 2  # contraction chunks covered by the lo-residual pass (4)
FP32 = mybir.dt.float32
BF16 = mybir.dt.bfloat16
FP16 = mybir.dt.float16
FP8 = mybir.dt.float8e4
EXP = mybir.ActivationFunctionType.Exp
ADD = mybir.AluOpType.add
SUB = mybir.AluOpType.subtract
DROW = mybir.MatmulPerfMode.DoubleRow
# logits scale: 1/sqrt(EMBED) softmax scale x 1/8 undoing the 8*Z prescale
SCALE = 1.0 / 256.0


def _build():
    nc = bacc.Bacc(
        "TRN2", target_bir_lowering=False, debug=False, num_devices=NCORES
    )
    xt_shard = nc.dram_tensor("xt_shard", [EMBED, M], FP16, kind="ExternalInput").ap()
    xt8_full = nc.dram_tensor(
        "xt8_full", [EMBED, N_TOKENS], FP8, kind="ExternalInput"
    ).ap()
    x16_full = nc.dram_tensor(
        "x16_full", [N_TOKENS, EMBED], FP16, kind="ExternalInput"
    ).ap()
    z_d = nc.dram_tensor("z", [EMBED, EMBED], FP16, kind="ExternalInput").ap()
    out_d = nc.dram_tensor("out", [M, EMBED], BF16, kind="ExternalOutput").ap()

    z_r = z_d.rearrange("(a p) d -> a p d", p=P)  # [DC, P, EMBED]
    xt_r = xt_shard.rearrange("(a p) m -> a p m", p=P)  # [DC, P, M]
    xt8_r = xt8_full.rearrange("(a p) n -> a p n", p=P)  # [DC, P, N]
    xv_r = x16_full.rearrange("(t p) d -> t p d", p=P)  # [64, P, EMBED]
    out_r = out_d.rearrange("(t p) d -> t p d", p=P)  # [MB, P, EMBED]

    with tile.TileContext(nc) as tc:
        with tc.tile_pool(name="persist", bufs=1) as pers:
            ones_sb = pers.tile([P, P], FP16)
            nc.vector.memset(ones_sb[:], 1.0)
            ident = pers.tile([P, P], FP32)
            make_identity(nc, ident[:])
            # q~^T resident as an fp8 hi+lo pair: qt_*[p, b*M + m]
            qt_hi = pers.tile([P, DC * M], FP8)
            qt_lo = pers.tile([P, DC * M], FP8)
            # fp32 PV accumulator per query block: [p, mb*EMBED + dv]
            out_acc = pers.tile([P, MB * EMBED], FP32)
            # softmax denominators, replicated across partitions: [p, m]
            sums_acc = pers.tile([P, M], FP32)

            # ---- Phase A: project q~^T = (8Z)^T @ x_own^T
            with (
                tc.tile_pool(name="proj", bufs=1) as proj,
                tc.tile_pool(name="proj_ps", bufs=4, space="PSUM") as proj_ps,
            ):
                z_sb = proj.tile([P, DC * EMBED], FP16)
                xt_sb = proj.tile([P, DC * M], FP16)
                for a in range(DC):
                    nc.sync.dma_start(
                        out=z_sb[:, a * EMBED : (a + 1) * EMBED], in_=z_r[a]
                    )
                    nc.sync.dma_start(
                        out=xt_sb[:, a * M : (a + 1) * M], in_=xt_r[a]
                    )
                for b in range(DC):  # output dim chunk
                    for j in range(M // NB):  # row half
                        ps = proj_ps.tile([P, NB], FP32, tag="proj_ps")
                        for a in range(DC):  # contraction chunk
                            nc.tensor.matmul(
                                ps[:],
                                lhsT=z_sb[:, a * EMBED + b * P : a * EMBED + (b + 1) * P],
                                rhs=xt_sb[:, a * M + j * NB : a * M + (j + 1) * NB],
                                start=(a == 0),
                                stop=(a == DC - 1),
                            )
                        sl = slice(b * M + j * NB, b * M + (j + 1) * NB)
                        nc.vector.tensor_copy(out=qt_hi[:, sl], in_=ps[:])
                        nc.vector.tensor_tensor(
                            out=qt_lo[:, sl], in0=ps[:], in1=qt_hi[:, sl], op=SUB
                        )

            qh_v = qt_hi[:].rearrange("p (b m) -> p b m", b=DC)  # [P, DC, M]
            ql_v = qt_lo[:].rearrange("p (b m) -> p b m", b=DC)

            # ---- Phase B: streaming attention over the 16 key blocks
            with (
                tc.tile_pool(name="kv", bufs=2) as kvp,
                tc.tile_pool(name="pb", bufs=3) as pbp,
                tc.tile_pool(name="ps_s", bufs=3, space="PSUM") as ps_sp,
                tc.tile_pool(name="ps_u", bufs=2, space="PSUM") as ps_up,
                tc.tile_pool(name="ps_o", bufs=2, space="PSUM") as ps_op,
                tc.tile_pool(name="fin", bufs=2) as fin,
            ):
                scol = fin.tile([P, MB], FP32)
                rtot = fin.tile([P, MB], FP32)
                for nb in range(NNB):
                    vtile = kvp.tile([P, VC * EMBED], FP16, tag="vtile")
                    for c in range(VC):
                        nc.sync.dma_start(
                            out=vtile[:, c * EMBED : (c + 1) * EMBED],
                            in_=xv_r[nb * VC + c],
                        )
                    ktile = kvp.tile([P, DC * NB], FP8, tag="ktile")
                    for b in range(DC):
                        nc.sync.dma_start(
                            out=ktile[:, b * NB : (b + 1) * NB],
                            in_=xt8_r[b, :, nb * NB : (nb + 1) * NB],
                        )
                    kt_v = ktile[:].rearrange("p (b n) -> p b n", b=DC)

                    pt_sb = pbp.tile([P, VC * M], FP16, tag="pt_sb")
                    for h in range(M // NB):  # query column half
                        for c in range(VC):  # key chunk within block
                            ps_s = ps_sp.tile([P, NB], FP32, tag="ps_s")
                            # fp8 DoubleRow: 2 contraction tiles/instruction;
                            # hi over all DC tiles, lo over LOC tiles whose
                            # position alternates per key chunk (error
                            # decorrelation across key sets).
                            lo0 = 0 if c % 2 == 0 else DC - LOC
                            for qi, q_v, b0, nbb in (
                                (0, qh_v, 0, DC // 2),
                                (1, ql_v, lo0, LOC // 2),
                            ):
                                for bb in range(nbb):
                                    s0 = b0 + 2 * bb
                                    nc.tensor.matmul(
                                        ps_s[:],
                                        lhsT=kt_v[
                                            :,
                                            s0 : s0 + 2,
                                            c * P : (c + 1) * P,
                                        ],
                                        rhs=q_v[
                                            :,
                                            s0 : s0 + 2,
                                            h * NB : (h + 1) * NB,
                                        ],
                                        start=(qi == 0 and bb == 0),
                                        stop=(qi == 1 and bb == LOC // 2 - 1),
                                        perf_mode=DROW,
                                    )
                            nc.scalar.activation(
                                out=pt_sb[:, c * M + h * NB : c * M + (h + 1) * NB],
                                in_=ps_s[:],
                                func=EXP,
                                scale=SCALE,
                            )
                    # partition-dim softmax sums via ones-vector matmul
                    for h in range(M // NB):
                        ps_sum = ps_up.tile([P, NB], FP32, tag="ps_sum")
                        for c in range(VC):
                            nc.tensor.matmul(
                                ps_sum[:],
                                lhsT=ones_sb[:],
                                rhs=pt_sb[:, c * M + h * NB : c * M + (h + 1) * NB],
                                start=(c == 0),
                                stop=(c == VC - 1),
                            )
                        dsts = sums_acc[:, h * NB : (h + 1) * NB]
                        if nb == 0:
                            nc.vector.tensor_copy(out=dsts, in_=ps_sum[:])
                        else:
                            nc.vector.tensor_tensor(
                                out=dsts, in0=dsts, in1=ps_sum[:], op=ADD
                            )
                    if nb == NNB - 1:
                        # softmax denominators are final: reciprocal per-query
                        # scalars now, so the divides pipeline with the last
                        # block's PV below.
                        for mb in range(MB):
                            ps_f = ps_up.tile([P, NB], FP32, tag="ps_sum")
                            nc.tensor.transpose(
                                out=ps_f[:, 0:P],
                                in_=sums_acc[:, mb * P : (mb + 1) * P],
                                identity=ident[:],
                            )
                            nc.vector.tensor_copy(
                                out=scol[:, mb : mb + 1], in_=ps_f[:, 0:1]
                            )
                        nc.vector.reciprocal(out=rtot[:], in_=scol[:])
                    for mb in range(MB):
                        for h in range(EMBED // NB):
                            ps_o = ps_op.tile([P, NB], FP32, tag="ps_o")
                            for t in range(VC):
                                nc.tensor.matmul(
                                    ps_o[:],
                                    lhsT=pt_sb[:, t * M + mb * P : t * M + (mb + 1) * P],
                                    rhs=vtile[:, t * EMBED + h * NB : t * EMBED + (h + 1) * NB],
                                    start=(t == 0),
                                    stop=(t == VC - 1),
                                )
                            dst = out_acc[:, mb * EMBED + h * NB : mb * EMBED + (h + 1) * NB]
                            if nb == 0:
                                nc.vector.tensor_copy(out=dst, in_=ps_o[:])
                            else:
                                nc.vector.tensor_tensor(
                                    out=dst, in0=dst, in1=ps_o[:], op=ADD
                                )
                        if nb == NNB - 1:
                            # this query block is complete: divide by the
                            # softmax sum and stream it out immediately
                            outf = fin.tile([P, EMBED], BF16, tag="outf")
                            nc.vector.tensor_scalar_mul(
                                outf[:],
                                out_acc[:, mb * EMBED : (mb + 1) * EMBED],
                                rtot[:, mb : mb + 1],
                            )
                            nc.sync.dma_start(out=out_r[mb], in_=outf[:])

    nc.compile()
    return nc


_NC = None


def _get_nc():
    global _NC
    if _NC is None:
        _NC = _build()
    return _NC


def _run(x, rotation_params, entangle_params, **spmd_kwargs):
    x = np.ascontiguousarray(np.asarray(x, dtype=np.float32))
    wq = np.asarray(rotation_params, dtype=np.float32).reshape(EMBED, EMBED)
    wk = np.asarray(entangle_params, dtype=np.float32).reshape(EMBED, EMBED)
    import ml_dtypes

    # offline weight folding: Z = 8 * Wq Wk^T (the 8x keeps the device-side
    # fp8 residual of q~ = x @ 8Z in e4m3's normal range; undone in the exp)
    z8 = (8.0 * (wq @ wk.T)).astype(np.float16)
    xt = np.ascontiguousarray(x.T)
    xt16 = xt.astype(np.float16)
    xt8 = xt.astype(ml_dtypes.float8_e4m3)
    x16 = x.astype(np.float16)
    in_maps = [
        {
            "xt_shard": np.ascontiguousarray(xt16[:, i * M : (i + 1) * M]),
            "xt8_full": xt8,
            "x16_full": x16,
            "z": z8,
        }
        for i in range(NCORES)
    ]
    res = bass_utils.run_bass_kernel_spmd(
        _get_nc(), in_maps, core_ids=list(range(NCORES)), **spmd_kwargs
    )
    out = np.concatenate(
        [res.results[i]["out"].astype(np.float32) for i in range(NCORES)], axis=0
    )
    return out, res


def kernel(x, rotation_params, entangle_params):
    out, _ = _run(x, rotation_params, entangle_params)
    return out


# revision 12
# speedup vs baseline: 1.5915x; 1.0422x over previous
"""Trainium2 Bass kernel for ClassicalSelfAttention.

  out = softmax((x @ Wq) @ (x @ Wk)^T / sqrt(D)) @ x      x: [8192, 1024] f32

Key algebraic restructuring: scores = (x Wq)(x Wk)^T = x (Wq Wk^T) x^T, so
the two weight matrices are folded offline (host-side, weight-only, input
independent) into Z = Wq Wk^T.  On device each core projects only its own
row-shard once (qt = x_own @ 8Z, the 8x pre-scale keeps the fp8 residual
in e4m3's normal range) and computes its scores row-block directly against
x^T -- which every core already holds as a kernel input for the PV stage.
This removes the K projection AND the K^T AllGather entirely: no
collective, no inter-core dependency, all 16 key blocks stream uniformly
from DRAM.

Sharding (8 NeuronCores): rows of x are sharded across cores; each core
runs a streaming attention loop over 16 key-blocks of 512 keys: fp8
scores matmul -> fused exp (with the 1/(32*8) logit scale) on ScalarE ->
fp16 PV matmul accumulated in SBUF fp32.  The softmax division is folded
into the final output scale.

The scores matmul runs in fp8 e4m3 with the DoubleRow perf mode (two
128-deep contraction tiles per instruction -> 2x MACs per PE cycle).  To
stay inside the correctness gate, q̃ is kept as an fp8 hi+lo pair
(lo = q̃ - fp8(q̃), re-quantized to fp8) and the scores accumulate
x8^T @ q_hi over all 1024 contraction dims plus a half-depth residual
x8^T @ q_lo over 512 dims (full depth would cost as much as bf16).
WHICH half the residual covers alternates per 128-key chunk: the two
coherent per-query error vectors from the uncovered halves then hit
disjoint key sets and add in quadrature instead of coherently, cutting
the residual error variance 2x at zero extra instructions.  The K-side
operand is host-quantized x^T in fp8.  The PV matmul runs fp16 with
fp32 PSUM.

The scores matmul keeps x^T stationary and q̃^T moving, so PSUM holds
scores TRANSPOSED ([key, query]); exp of that is P^T directly -- exactly
the layout the PV matmul needs as its stationary operand -- so no PE
transposes are needed.  The softmax row-sums (a partition-dim reduction
in this layout) are computed by a ones-vector matmul and fixed up into
per-partition scalars at the end.
"""

import sys

import numpy as np

try:
    import concourse.bass as bass  # noqa: F401
except ImportError:  # pragma: no cover
    sys.path.insert(0, "/opt/trn_rl_repo")

import concourse.bacc as bacc
import concourse.mybir as mybir
import concourse.tile as tile
from concourse.masks import make_identity
from concourse import bass_utils

N_TOKENS = 8192
EMBED = 1024
NCORES = 8
M = N_TOKENS // NCORES  # rows per core (1024)
P = 128  # partitions
DC = EMBED // P  # contraction chunks (8)
NB = 512  # key-block width
NNB = N_TOKENS // NB  # key blocks (16)
MB = M // P  # query row-blocks per core (8)
VC = NB // P  # value chunks per key block (4)
LOC = DC // 2  # contraction chunks covered by the lo-residual pass (4)
FP32 = mybir.dt.float32
BF16 = mybir.dt.bfloat16
FP16 = mybir.dt.float16
FP8 = mybir.dt.float8e4
EXP = mybir.ActivationFunctionType.Exp
ADD = mybir.AluOpType.add
SUB = mybir.AluOpType.subtract
DROW = mybir.MatmulPerfMode.DoubleRow
# logits scale: 1/sqrt(EMBED) softmax scale x 1/8 undoing the 8*Z prescale
SCALE = 1.0 / 256.0


def _build():
    nc = bacc.Bacc(
        "TRN2", target_bir_lowering=False, debug=False, num_devices=NCORES
    )
    xt_shard = nc.dram_tensor("xt_shard", [EMBED, M], FP16, kind="ExternalInput").ap()
    xt8_full = nc.dram_tensor(
        "xt8_full", [EMBED, N_TOKENS], FP8, kind="ExternalInput"
    ).ap()
    x16_full = nc.dram_tensor(
        "x16_full", [N_TOKENS, EMBED], FP16, kind="ExternalInput"
    ).ap()
    z_d = nc.dram_tensor("z", [EMBED, EMBED], FP16, kind="ExternalInput").ap()
    out_d = nc.dram_tensor("out", [M, EMBED], BF16, kind="ExternalOutput").ap()

    z_r = z_d.rearrange("(a p) d -> a p d", p=P)  # [DC, P, EMBED]
    xt_r = xt_shard.rearrange("(a p) m -> a p m", p=P)  # [DC, P, M]
    xt8_r = xt8_full.rearrange("(a p) n -> a p n", p=P)  # [DC, P, N]
    xv_r = x16_full.rearrange("(t p) d -> t p d", p=P)  # [64, P, EMBED]
    out_r = out_d.rearrange("(t p) d -> t p d", p=P)  # [MB, P, EMBED]

    with tile.TileContext(nc) as tc:
        with tc.tile_pool(name="persist", bufs=1) as pers:
            ones8 = pers.tile([P, 2 * P], FP8)
            nc.vector.memset(ones8[:], 1.0)
            ident = pers.tile([P, P], FP32)
            make_identity(nc, ident[:])
            # q~^T resident as an fp8 hi+lo pair: qt_*[p, b*M + m]
            qt_hi = pers.tile([P, DC * M], FP8)
            qt_lo = pers.tile([P, DC * M], FP8)
            # fp32 PV accumulator per query block: [p, mb*EMBED + dv]
            out_acc = pers.tile([P, MB * EMBED], FP32)
            # softmax denominators, replicated across partitions: [p, m]
            sums_acc = pers.tile([P, M], FP32)

            # ---- Phase A: project q~^T = (8Z)^T @ x_own^T
            with (
                tc.tile_pool(name="proj", bufs=1) as proj,
                tc.tile_pool(name="proj_ps", bufs=4, space="PSUM") as proj_ps,
            ):
                z_sb = proj.tile([P, DC * EMBED], FP16)
                xt_sb = proj.tile([P, DC * M], FP16)
                for a in range(DC):
                    nc.sync.dma_start(
                        out=z_sb[:, a * EMBED : (a + 1) * EMBED], in_=z_r[a]
                    )
                    nc.sync.dma_start(
                        out=xt_sb[:, a * M : (a + 1) * M], in_=xt_r[a]
                    )
                for b in range(DC):  # output dim chunk
                    for j in range(M // NB):  # row half
                        ps = proj_ps.tile([P, NB], FP32, tag="proj_ps")
                        for a in range(DC):  # contraction chunk
                            nc.tensor.matmul(
                                ps[:],
                                lhsT=z_sb[:, a * EMBED + b * P : a * EMBED + (b + 1) * P],
                                rhs=xt_sb[:, a * M + j * NB : a * M + (j + 1) * NB],
                                start=(a == 0),
                                stop=(a == DC - 1),
                            )
                        sl = slice(b * M + j * NB, b * M + (j + 1) * NB)
                        nc.vector.tensor_copy(out=qt_hi[:, sl], in_=ps[:])
                        nc.vector.tensor_tensor(
                            out=qt_lo[:, sl], in0=ps[:], in1=qt_hi[:, sl], op=SUB
                        )

            qh_v = qt_hi[:].rearrange("p (b m) -> p b m", b=DC)  # [P, DC, M]
            ql_v = qt_lo[:].rearrange("p (b m) -> p b m", b=DC)

            # ---- Phase B: streaming attention over the 16 key blocks
            with (
                tc.tile_pool(name="kv", bufs=2) as kvp,
                tc.tile_pool(name="pb", bufs=2) as pbp,
                tc.tile_pool(name="ps_s", bufs=3, space="PSUM") as ps_sp,
                tc.tile_pool(name="ps_u", bufs=2, space="PSUM") as ps_up,
                tc.tile_pool(name="ps_o", bufs=2, space="PSUM") as ps_op,
                tc.tile_pool(name="fin", bufs=2) as fin,
            ):
                scol = fin.tile([P, MB], FP32)
                rtot = fin.tile([P, MB], FP32)
                ones2_v = ones8[:].rearrange("p (s q) -> p s q", s=2)
                # process key blocks in PAIRS: PV and sums accumulate both
                # blocks of a pair inside one PSUM group, halving the DVE
                # accumulate traffic into out_acc/sums_acc.
                for np_ in range(NNB // 2):
                    pts, pt8s, vts = [], [], []
                    for blk in range(2):
                        nb = 2 * np_ + blk
                        vtile = kvp.tile([P, VC * EMBED], FP16, tag=f"vt{blk}")
                        for c in range(VC):
                            nc.sync.dma_start(
                                out=vtile[:, c * EMBED : (c + 1) * EMBED],
                                in_=xv_r[nb * VC + c],
                            )
                        ktile = kvp.tile([P, DC * NB], FP8, tag=f"kt{blk}")
                        for b in range(DC):
                            nc.sync.dma_start(
                                out=ktile[:, b * NB : (b + 1) * NB],
                                in_=xt8_r[b, :, nb * NB : (nb + 1) * NB],
                            )
                        kt_v = ktile[:].rearrange("p (b n) -> p b n", b=DC)

                        pt_sb = pbp.tile([P, VC * M], FP16, tag=f"pt{blk}")
                        pt8_sb = pbp.tile([P, VC * M], FP8, tag=f"pt8_{blk}")
                        for h in range(M // NB):  # query column half
                            for c in range(VC):  # key chunk within block
                                ps_s = ps_sp.tile([P, NB], FP32, tag="ps_s")
                                # fp8 DoubleRow: 2 contraction tiles per
                                # instruction; hi over all DC tiles, lo over
                                # LOC tiles whose position alternates per key
                                # chunk (error decorrelation across key sets).
                                lo0 = 0 if c % 2 == 0 else DC - LOC
                                for qi, q_v, b0, nbb in (
                                    (0, qh_v, 0, DC // 2),
                                    (1, ql_v, lo0, LOC // 2),
                                ):
                                    for bb in range(nbb):
                                        s0 = b0 + 2 * bb
                                        nc.tensor.matmul(
                                            ps_s[:],
                                            lhsT=kt_v[
                                                :,
                                                s0 : s0 + 2,
                                                c * P : (c + 1) * P,
                                            ],
                                            rhs=q_v[
                                                :,
                                                s0 : s0 + 2,
                                                h * NB : (h + 1) * NB,
                                            ],
                                            start=(qi == 0 and bb == 0),
                                            stop=(qi == 1 and bb == LOC // 2 - 1),
                                            perf_mode=DROW,
                                        )
                                csl = slice(c * M + h * NB, c * M + (h + 1) * NB)
                                nc.scalar.activation(
                                    out=pt_sb[:, csl], in_=ps_s[:],
                                    func=EXP, scale=SCALE,
                                )
                                # fp8 copy of P^T feeds the DoubleRow row-sums
                                nc.scalar.activation(
                                    out=pt8_sb[:, csl], in_=ps_s[:],
                                    func=EXP, scale=SCALE,
                                )
                        pts.append(pt_sb)
                        pt8s.append(pt8_sb)
                        vts.append(vtile)

                    # partition-dim softmax sums: fp8 DoubleRow ones-matmul
                    # over both blocks of the pair (2 key chunks/instruction)
                    for h in range(M // NB):
                        ps_sum = ps_up.tile([P, NB], FP32, tag="ps_sum")
                        for blk in range(2):
                            p8_v = pt8s[blk][:].rearrange("p (c m) -> p c m", c=VC)
                            for cc in range(VC // 2):
                                nc.tensor.matmul(
                                    ps_sum[:],
                                    lhsT=ones2_v,
                                    rhs=p8_v[
                                        :, 2 * cc : 2 * cc + 2, h * NB : (h + 1) * NB
                                    ],
                                    start=(blk == 0 and cc == 0),
                                    stop=(blk == 1 and cc == VC // 2 - 1),
                                    perf_mode=DROW,
                                )
                        dsts = sums_acc[:, h * NB : (h + 1) * NB]
                        if np_ == 0:
                            nc.vector.tensor_copy(out=dsts, in_=ps_sum[:])
                        else:
                            nc.vector.tensor_tensor(
                                out=dsts, in0=dsts, in1=ps_sum[:], op=ADD
                            )
                    if np_ == NNB // 2 - 1:
                        # softmax denominators are final: reciprocal per-query
                        # scalars now, so the divides pipeline with the last
                        # pair's PV below.
                        for mb in range(MB):
                            ps_f = ps_up.tile([P, NB], FP32, tag="ps_sum")
                            nc.tensor.transpose(
                                out=ps_f[:, 0:P],
                                in_=sums_acc[:, mb * P : (mb + 1) * P],
                                identity=ident[:],
                            )
                            nc.vector.tensor_copy(
                                out=scol[:, mb : mb + 1], in_=ps_f[:, 0:1]
                            )
                        nc.vector.reciprocal(out=rtot[:], in_=scol[:])
                    for mb in range(MB):
                        for h in range(EMBED // NB):
                            ps_o = ps_op.tile([P, NB], FP32, tag="ps_o")
                            for blk in range(2):
                                for t in range(VC):
                                    nc.tensor.matmul(
                                        ps_o[:],
                                        lhsT=pts[blk][:, t * M + mb * P : t * M + (mb + 1) * P],
                                        rhs=vts[blk][:, t * EMBED + h * NB : t * EMBED + (h + 1) * NB],
                                        start=(blk == 0 and t == 0),
                                        stop=(blk == 1 and t == VC - 1),
                                    )
                            dst = out_acc[:, mb * EMBED + h * NB : mb * EMBED + (h + 1) * NB]
                            if np_ == 0:
                                nc.vector.tensor_copy(out=dst, in_=ps_o[:])
                            else:
                                nc.vector.tensor_tensor(
                                    out=dst, in0=dst, in1=ps_o[:], op=ADD
                                )
                        if np_ == NNB // 2 - 1:
                            # this query block is complete: divide by the
                            # softmax sum and stream it out immediately
                            outf = fin.tile([P, EMBED], BF16, tag="outf")
                            nc.vector.tensor_scalar_mul(
                                outf[:],
                                out_acc[:, mb * EMBED : (mb + 1) * EMBED],
                                rtot[:, mb : mb + 1],
                            )
                            nc.sync.dma_start(out=out_r[mb], in_=outf[:])

    nc.compile()
    return nc


_NC = None


def _get_nc():
    global _NC
    if _NC is None:
        _NC = _build()
    return _NC


def _run(x, rotation_params, entangle_params, **spmd_kwargs):
    x = np.ascontiguousarray(np.asarray(x, dtype=np.float32))
    wq = np.asarray(rotation_params, dtype=np.float32).reshape(EMBED, EMBED)
    wk = np.asarray(entangle_params, dtype=np.float32).reshape(EMBED, EMBED)
    import ml_dtypes

    # offline weight folding: Z = 8 * Wq Wk^T (the 8x keeps the device-side
    # fp8 residual of q~ = x @ 8Z in e4m3's normal range; undone in the exp)
    z8 = (8.0 * (wq @ wk.T)).astype(np.float16)
    xt = np.ascontiguousarray(x.T)
    xt16 = xt.astype(np.float16)
    xt8 = xt.astype(ml_dtypes.float8_e4m3)
    x16 = x.astype(np.float16)
    in_maps = [
        {
            "xt_shard": np.ascontiguousarray(xt16[:, i * M : (i + 1) * M]),
            "xt8_full": xt8,
            "x16_full": x16,
            "z": z8,
        }
        for i in range(NCORES)
    ]
    res = bass_utils.run_bass_kernel_spmd(
        _get_nc(), in_maps, core_ids=list(range(NCORES)), **spmd_kwargs
    )
    out = np.concatenate(
        [res.results[i]["out"].astype(np.float32) for i in range(NCORES)], axis=0
    )
    return out, res


def kernel(x, rotation_params, entangle_params):
    out, _ = _run(x, rotation_params, entangle_params)
    return out


# revision 13
# speedup vs baseline: 1.5947x; 1.0020x over previous
"""Trainium2 Bass kernel for ClassicalSelfAttention.

  out = softmax((x @ Wq) @ (x @ Wk)^T / sqrt(D)) @ x      x: [8192, 1024] f32

Key algebraic restructuring: scores = (x Wq)(x Wk)^T = x (Wq Wk^T) x^T, so
the two weight matrices are folded offline (host-side, weight-only, input
independent) into Z = Wq Wk^T.  On device each core projects only its own
row-shard once (qt = x_own @ 8Z, the 8x pre-scale keeps the fp8 residual
in e4m3's normal range) and computes its scores row-block directly against
x^T -- which every core already holds as a kernel input for the PV stage.
This removes the K projection AND the K^T AllGather entirely: no
collective, no inter-core dependency, all 16 key blocks stream uniformly
from DRAM.

Sharding (8 NeuronCores): rows of x are sharded across cores; each core
runs a streaming attention loop over 16 key-blocks of 512 keys: fp8
scores matmul -> fused exp (with the 1/(32*8) logit scale) on ScalarE ->
fp16 PV matmul accumulated in SBUF fp32.  The softmax division is folded
into the final output scale.

The scores matmul runs in fp8 e4m3 with the DoubleRow perf mode (two
128-deep contraction tiles per instruction -> 2x MACs per PE cycle).  To
stay inside the correctness gate, q̃ is kept as an fp8 hi+lo pair
(lo = q̃ - fp8(q̃), re-quantized to fp8) and the scores accumulate
x8^T @ q_hi over all 1024 contraction dims plus a half-depth residual
x8^T @ q_lo over 512 dims (full depth would cost as much as bf16).
WHICH half the residual covers alternates per 128-key chunk: the two
coherent per-query error vectors from the uncovered halves then hit
disjoint key sets and add in quadrature instead of coherently, cutting
the residual error variance 2x at zero extra instructions.  The K-side
operand is host-quantized x^T in fp8.  The PV matmul runs fp16 with
fp32 PSUM.

The scores matmul keeps x^T stationary and q̃^T moving, so PSUM holds
scores TRANSPOSED ([key, query]); exp of that is P^T directly -- exactly
the layout the PV matmul needs as its stationary operand -- so no PE
transposes are needed.  The softmax row-sums (a partition-dim reduction
in this layout) are computed by a ones-vector matmul and fixed up into
per-partition scalars at the end.
"""

import sys

import numpy as np

try:
    import concourse.bass as bass  # noqa: F401
except ImportError:  # pragma: no cover
    sys.path.insert(0, "/opt/trn_rl_repo")

import concourse.bacc as bacc
import concourse.mybir as mybir
import concourse.tile as tile
from concourse.masks import make_identity
from concourse import bass_utils

N_TOKENS = 8192
EMBED = 1024
NCORES = 8
M = N_TOKENS // NCORES  # rows per core (1024)
P = 128  # partitions
DC = EMBED // P  # contraction chunks (8)
NB = 512  # key-block width
NNB = N_TOKENS // NB  # key blocks (16)
MB = M // P  # query row-blocks per core (8)
VC = NB // P  # value chunks per key block (4)
LOC = DC // 2  # contraction chunks covered by the lo-residual pass (4)
FP32 = mybir.dt.float32
BF16 = mybir.dt.bfloat16
FP16 = mybir.dt.float16
FP8 = mybir.dt.float8e4
EXP = mybir.ActivationFunctionType.Exp
ADD = mybir.AluOpType.add
SUB = mybir.AluOpType.subtract
DROW = mybir.MatmulPerfMode.DoubleRow
# logits scale: 1/sqrt(EMBED) softmax scale x 1/8 undoing the 8*Z prescale
SCALE = 1.0 / 256.0


def _build():
    nc = bacc.Bacc(
        "TRN2", target_bir_lowering=False, debug=False, num_devices=NCORES
    )
    xt_shard = nc.dram_tensor("xt_shard", [EMBED, M], FP16, kind="ExternalInput").ap()
    xt8_full = nc.dram_tensor(
        "xt8_full", [EMBED, N_TOKENS], FP8, kind="ExternalInput"
    ).ap()
    x16_full = nc.dram_tensor(
        "x16_full", [N_TOKENS, EMBED], FP16, kind="ExternalInput"
    ).ap()
    z_d = nc.dram_tensor("z", [EMBED, EMBED], FP16, kind="ExternalInput").ap()
    out_d = nc.dram_tensor("out", [M, EMBED], BF16, kind="ExternalOutput").ap()

    z_r = z_d.rearrange("(a p) d -> a p d", p=P)  # [DC, P, EMBED]
    xt_r = xt_shard.rearrange("(a p) m -> a p m", p=P)  # [DC, P, M]
    xt8_r = xt8_full.rearrange("(a p) n -> a p n", p=P)  # [DC, P, N]
    xv_r = x16_full.rearrange("(t p) d -> t p d", p=P)  # [64, P, EMBED]
    out_r = out_d.rearrange("(t p) d -> t p d", p=P)  # [MB, P, EMBED]

    with tile.TileContext(nc) as tc:
        with tc.tile_pool(name="persist", bufs=1) as pers:
            ones8 = pers.tile([P, 2 * P], FP8)
            nc.vector.memset(ones8[:], 1.0)
            ident = pers.tile([P, P], FP32)
            make_identity(nc, ident[:])
            # q~^T resident as an fp8 hi+lo pair: qt_*[p, b*M + m]
            qt_hi = pers.tile([P, DC * M], FP8)
            qt_lo = pers.tile([P, DC * M], FP8)
            # fp32 PV accumulator per query block: [p, mb*EMBED + dv]
            out_acc = pers.tile([P, MB * EMBED], FP32)
            # softmax denominators, replicated across partitions: [p, m]
            sums_acc = pers.tile([P, M], FP32)

            # ---- Phase A: project q~^T = (8Z)^T @ x_own^T
            with (
                tc.tile_pool(name="proj", bufs=1) as proj,
                tc.tile_pool(name="proj_ps", bufs=4, space="PSUM") as proj_ps,
            ):
                z_sb = proj.tile([P, DC * EMBED], FP16)
                xt_sb = proj.tile([P, DC * M], FP16)
                for a in range(DC):
                    nc.sync.dma_start(
                        out=z_sb[:, a * EMBED : (a + 1) * EMBED], in_=z_r[a]
                    )
                    nc.sync.dma_start(
                        out=xt_sb[:, a * M : (a + 1) * M], in_=xt_r[a]
                    )
                for b in range(DC):  # output dim chunk
                    for j in range(M // NB):  # row half
                        ps = proj_ps.tile([P, NB], FP32, tag="proj_ps")
                        for a in range(DC):  # contraction chunk
                            nc.tensor.matmul(
                                ps[:],
                                lhsT=z_sb[:, a * EMBED + b * P : a * EMBED + (b + 1) * P],
                                rhs=xt_sb[:, a * M + j * NB : a * M + (j + 1) * NB],
                                start=(a == 0),
                                stop=(a == DC - 1),
                            )
                        sl = slice(b * M + j * NB, b * M + (j + 1) * NB)
                        nc.vector.tensor_copy(out=qt_hi[:, sl], in_=ps[:])
                        nc.vector.tensor_tensor(
                            out=qt_lo[:, sl], in0=ps[:], in1=qt_hi[:, sl], op=SUB
                        )

            qh_v = qt_hi[:].rearrange("p (b m) -> p b m", b=DC)  # [P, DC, M]
            ql_v = qt_lo[:].rearrange("p (b m) -> p b m", b=DC)

            # ---- Phase B: streaming attention over the 16 key blocks
            with (
                tc.tile_pool(name="kv", bufs=2) as kvp,
                tc.tile_pool(name="pb", bufs=2) as pbp,
                tc.tile_pool(name="ps_s", bufs=3, space="PSUM") as ps_sp,
                tc.tile_pool(name="ps_u", bufs=2, space="PSUM") as ps_up,
                tc.tile_pool(name="ps_o", bufs=2, space="PSUM") as ps_op,
                tc.tile_pool(name="fin", bufs=2) as fin,
            ):
                scol = fin.tile([P, MB], FP32)
                rtot = fin.tile([P, MB], FP32)
                ones2_v = ones8[:].rearrange("p (s q) -> p s q", s=2)
                # process key blocks in PAIRS: PV and sums accumulate both
                # blocks of a pair inside one PSUM group, halving the DVE
                # accumulate traffic into out_acc/sums_acc.
                for np_ in range(NNB // 2):
                    pts, pt8s, vts = [], [], []
                    for blk in range(2):
                        nb = 2 * np_ + blk
                        vtile = kvp.tile([P, VC * EMBED], FP16, tag=f"vt{blk}")
                        for c in range(VC):
                            nc.sync.dma_start(
                                out=vtile[:, c * EMBED : (c + 1) * EMBED],
                                in_=xv_r[nb * VC + c],
                            )
                        ktile = kvp.tile([P, DC * NB], FP8, tag=f"kt{blk}")
                        for b in range(DC):
                            nc.sync.dma_start(
                                out=ktile[:, b * NB : (b + 1) * NB],
                                in_=xt8_r[b, :, nb * NB : (nb + 1) * NB],
                            )
                        kt_v = ktile[:].rearrange("p (b n) -> p b n", b=DC)

                        pt_sb = pbp.tile([P, VC * M], FP16, tag=f"pt{blk}")
                        pt8_sb = pbp.tile([P, VC * M], FP8, tag=f"pt8_{blk}")
                        for h in range(M // NB):  # query column half
                            for c in range(VC):  # key chunk within block
                                ps_s = ps_sp.tile([P, NB], FP32, tag="ps_s")
                                # fp8 DoubleRow: 2 contraction tiles per
                                # instruction; hi over all DC tiles, lo over
                                # LOC tiles whose position alternates per key
                                # chunk (error decorrelation across key sets).
                                lo0 = 0 if c % 2 == 0 else DC - LOC
                                for qi, q_v, b0, nbb in (
                                    (0, qh_v, 0, DC // 2),
                                    (1, ql_v, lo0, LOC // 2),
                                ):
                                    for bb in range(nbb):
                                        s0 = b0 + 2 * bb
                                        nc.tensor.matmul(
                                            ps_s[:],
                                            lhsT=kt_v[
                                                :,
                                                s0 : s0 + 2,
                                                c * P : (c + 1) * P,
                                            ],
                                            rhs=q_v[
                                                :,
                                                s0 : s0 + 2,
                                                h * NB : (h + 1) * NB,
                                            ],
                                            start=(qi == 0 and bb == 0),
                                            stop=(qi == 1 and bb == LOC // 2 - 1),
                                            perf_mode=DROW,
                                        )
                                csl = slice(c * M + h * NB, c * M + (h + 1) * NB)
                                nc.scalar.activation(
                                    out=pt_sb[:, csl], in_=ps_s[:],
                                    func=EXP, scale=SCALE,
                                )
                                # fp8 copy of P^T feeds the DoubleRow row-sums
                                # (on DVE so ScalarE stays off the critical path)
                                nc.vector.tensor_copy(
                                    out=pt8_sb[:, csl], in_=pt_sb[:, csl]
                                )
                        pts.append(pt_sb)
                        pt8s.append(pt8_sb)
                        vts.append(vtile)

                    # partition-dim softmax sums: fp8 DoubleRow ones-matmul
                    # over both blocks of the pair (2 key chunks/instruction)
                    for h in range(M // NB):
                        ps_sum = ps_up.tile([P, NB], FP32, tag="ps_sum")
                        for blk in range(2):
                            p8_v = pt8s[blk][:].rearrange("p (c m) -> p c m", c=VC)
                            for cc in range(VC // 2):
                                nc.tensor.matmul(
                                    ps_sum[:],
                                    lhsT=ones2_v,
                                    rhs=p8_v[
                                        :, 2 * cc : 2 * cc + 2, h * NB : (h + 1) * NB
                                    ],
                                    start=(blk == 0 and cc == 0),
                                    stop=(blk == 1 and cc == VC // 2 - 1),
                                    perf_mode=DROW,
                                )
                        dsts = sums_acc[:, h * NB : (h + 1) * NB]
                        if np_ == 0:
                            nc.vector.tensor_copy(out=dsts, in_=ps_sum[:])
                        else:
                            nc.vector.tensor_tensor(
                                out=dsts, in0=dsts, in1=ps_sum[:], op=ADD
                            )
                    if np_ == NNB // 2 - 1:
                        # softmax denominators are final: reciprocal per-query
                        # scalars now, so the divides pipeline with the last
                        # pair's PV below.
                        for mb in range(MB):
                            ps_f = ps_up.tile([P, NB], FP32, tag="ps_sum")
                            nc.tensor.transpose(
                                out=ps_f[:, 0:P],
                                in_=sums_acc[:, mb * P : (mb + 1) * P],
                                identity=ident[:],
                            )
                            nc.vector.tensor_copy(
                                out=scol[:, mb : mb + 1], in_=ps_f[:, 0:1]
                            )
                        nc.vector.reciprocal(out=rtot[:], in_=scol[:])
                    for mb in range(MB):
                        for h in range(EMBED // NB):
                            ps_o = ps_op.tile([P, NB], FP32, tag="ps_o")
                            for blk in range(2):
                                for t in range(VC):
                                    nc.tensor.matmul(
                                        ps_o[:],
                                        lhsT=pts[blk][:, t * M + mb * P : t * M + (mb + 1) * P],
                                        rhs=vts[blk][:, t * EMBED + h * NB : t * EMBED + (h + 1) * NB],
                                        start=(blk == 0 and t == 0),
                                        stop=(blk == 1 and t == VC - 1),
                                    )
                            dst = out_acc[:, mb * EMBED + h * NB : mb * EMBED + (h + 1) * NB]
                            if np_ == 0:
                                nc.vector.tensor_copy(out=dst, in_=ps_o[:])
                            else:
                                nc.vector.tensor_tensor(
                                    out=dst, in0=dst, in1=ps_o[:], op=ADD
                                )
                        if np_ == NNB // 2 - 1:
                            # this query block is complete: divide by the
                            # softmax sum and stream it out immediately
                            outf = fin.tile([P, EMBED], BF16, tag="outf")
                            nc.vector.tensor_scalar_mul(
                                outf[:],
                                out_acc[:, mb * EMBED : (mb + 1) * EMBED],
                                rtot[:, mb : mb + 1],
                            )
                            nc.sync.dma_start(out=out_r[mb], in_=outf[:])

    nc.compile()
    return nc


_NC = None


def _get_nc():
    global _NC
    if _NC is None:
        _NC = _build()
    return _NC


def _run(x, rotation_params, entangle_params, **spmd_kwargs):
    x = np.ascontiguousarray(np.asarray(x, dtype=np.float32))
    wq = np.asarray(rotation_params, dtype=np.float32).reshape(EMBED, EMBED)
    wk = np.asarray(entangle_params, dtype=np.float32).reshape(EMBED, EMBED)
    import ml_dtypes

    # offline weight folding: Z = 8 * Wq Wk^T (the 8x keeps the device-side
    # fp8 residual of q~ = x @ 8Z in e4m3's normal range; undone in the exp)
    z8 = (8.0 * (wq @ wk.T)).astype(np.float16)
    xt = np.ascontiguousarray(x.T)
    xt16 = xt.astype(np.float16)
    xt8 = xt.astype(ml_dtypes.float8_e4m3)
    x16 = x.astype(np.float16)
    in_maps = [
        {
            "xt_shard": np.ascontiguousarray(xt16[:, i * M : (i + 1) * M]),
            "xt8_full": xt8,
            "x16_full": x16,
            "z": z8,
        }
        for i in range(NCORES)
    ]
    res = bass_utils.run_bass_kernel_spmd(
        _get_nc(), in_maps, core_ids=list(range(NCORES)), **spmd_kwargs
    )
    out = np.concatenate(
        [res.results[i]["out"].astype(np.float32) for i in range(NCORES)], axis=0
    )
    return out, res


def kernel(x, rotation_params, entangle_params):
    out, _ = _run(x, rotation_params, entangle_params)
    return out


# revision 14
# speedup vs baseline: 1.5998x; 1.0032x over previous
"""Trainium2 Bass kernel for ClassicalSelfAttention.

  out = softmax((x @ Wq) @ (x @ Wk)^T / sqrt(D)) @ x      x: [8192, 1024] f32

Key algebraic restructuring: scores = (x Wq)(x Wk)^T = x (Wq Wk^T) x^T, so
the two weight matrices are folded offline (host-side, weight-only, input
independent) into Z = Wq Wk^T.  On device each core projects only its own
row-shard once (qt = x_own @ 8Z, the 8x pre-scale keeps the fp8 residual
in e4m3's normal range) and computes its scores row-block directly against
x^T -- which every core already holds as a kernel input for the PV stage.
This removes the K projection AND the K^T AllGather entirely: no
collective, no inter-core dependency, all 16 key blocks stream uniformly
from DRAM.

Sharding (8 NeuronCores): rows of x are sharded across cores; each core
runs a streaming attention loop over 16 key-blocks of 512 keys: fp8
scores matmul -> fused exp (with the 1/(32*8) logit scale) on ScalarE ->
fp16 PV matmul accumulated in SBUF fp32.  The softmax division is folded
into the final output scale.

The scores matmul runs in fp8 e4m3 with the DoubleRow perf mode (two
128-deep contraction tiles per instruction -> 2x MACs per PE cycle).  To
stay inside the correctness gate, q̃ is kept as an fp8 hi+lo pair
(lo = q̃ - fp8(q̃), re-quantized to fp8) and the scores accumulate
x8^T @ q_hi over all 1024 contraction dims plus a half-depth residual
x8^T @ q_lo over 512 dims (full depth would cost as much as bf16).
WHICH half the residual covers alternates per 128-key chunk: the two
coherent per-query error vectors from the uncovered halves then hit
disjoint key sets and add in quadrature instead of coherently, cutting
the residual error variance 2x at zero extra instructions.  The K-side
operand is host-quantized x^T in fp8.  The PV matmul runs fp16 with
fp32 PSUM.

The scores matmul keeps x^T stationary and q̃^T moving, so PSUM holds
scores TRANSPOSED ([key, query]); exp of that is P^T directly -- exactly
the layout the PV matmul needs as its stationary operand -- so no PE
transposes are needed.  The softmax row-sums (a partition-dim reduction
in this layout) are computed by an fp8 DoubleRow ones-vector matmul on an fp8
copy of P^T and fixed up into per-partition scalars at the end; PV and
the row-sums accumulate block PAIRS in single PSUM groups to halve the
DVE accumulate traffic.
"""

import sys

import numpy as np

try:
    import concourse.bass as bass  # noqa: F401
except ImportError:  # pragma: no cover
    sys.path.insert(0, "/opt/trn_rl_repo")

import concourse.bacc as bacc
import concourse.mybir as mybir
import concourse.tile as tile
from concourse.masks import make_identity
from concourse import bass_utils

N_TOKENS = 8192
EMBED = 1024
NCORES = 8
M = N_TOKENS // NCORES  # rows per core (1024)
P = 128  # partitions
DC = EMBED // P  # contraction chunks (8)
NB = 512  # key-block width
NNB = N_TOKENS // NB  # key blocks (16)
MB = M // P  # query row-blocks per core (8)
VC = NB // P  # value chunks per key block (4)
LOC = DC // 2  # contraction chunks covered by the lo-residual pass (4)
FP32 = mybir.dt.float32
BF16 = mybir.dt.bfloat16
FP16 = mybir.dt.float16
FP8 = mybir.dt.float8e4
EXP = mybir.ActivationFunctionType.Exp
ADD = mybir.AluOpType.add
SUB = mybir.AluOpType.subtract
DROW = mybir.MatmulPerfMode.DoubleRow
# logits scale: 1/sqrt(EMBED) softmax scale x 1/8 undoing the 8*Z prescale
SCALE = 1.0 / 256.0


def _build():
    nc = bacc.Bacc(
        "TRN2", target_bir_lowering=False, debug=False, num_devices=NCORES
    )
    xt_shard = nc.dram_tensor("xt_shard", [EMBED, M], FP16, kind="ExternalInput").ap()
    xt8_full = nc.dram_tensor(
        "xt8_full", [EMBED, N_TOKENS], FP8, kind="ExternalInput"
    ).ap()
    x16_full = nc.dram_tensor(
        "x16_full", [N_TOKENS, EMBED], FP16, kind="ExternalInput"
    ).ap()
    z_d = nc.dram_tensor("z", [EMBED, EMBED], FP16, kind="ExternalInput").ap()
    out_d = nc.dram_tensor("out", [M, EMBED], BF16, kind="ExternalOutput").ap()

    z_r = z_d.rearrange("(a p) d -> a p d", p=P)  # [DC, P, EMBED]
    xt_r = xt_shard.rearrange("(a p) m -> a p m", p=P)  # [DC, P, M]
    xt8_r = xt8_full.rearrange("(a p) n -> a p n", p=P)  # [DC, P, N]
    xv_r = x16_full.rearrange("(t p) d -> t p d", p=P)  # [64, P, EMBED]
    out_r = out_d.rearrange("(t p) d -> t p d", p=P)  # [MB, P, EMBED]

    with tile.TileContext(nc) as tc:
        with tc.tile_pool(name="persist", bufs=1) as pers:
            ones8 = pers.tile([P, 2 * P], FP8)
            nc.vector.memset(ones8[:], 1.0)
            ident = pers.tile([P, P], FP32)
            make_identity(nc, ident[:])
            # q~^T resident as an fp8 hi+lo pair: qt_*[p, b*M + m]
            qt_hi = pers.tile([P, DC * M], FP8)
            qt_lo = pers.tile([P, DC * M], FP8)
            # fp32 PV accumulator per query block: [p, mb*EMBED + dv]
            out_acc = pers.tile([P, MB * EMBED], FP32)
            # softmax denominators, replicated across partitions: [p, m]
            sums_acc = pers.tile([P, M], FP32)

            # ---- Phase A: project q~^T = (8Z)^T @ x_own^T
            with (
                tc.tile_pool(name="proj", bufs=1) as proj,
                tc.tile_pool(name="proj_ps", bufs=4, space="PSUM") as proj_ps,
            ):
                z_sb = proj.tile([P, DC * EMBED], FP16)
                xt_sb = proj.tile([P, DC * M], FP16)
                for a in range(DC):
                    nc.sync.dma_start(
                        out=z_sb[:, a * EMBED : (a + 1) * EMBED], in_=z_r[a]
                    )
                    nc.sync.dma_start(
                        out=xt_sb[:, a * M : (a + 1) * M], in_=xt_r[a]
                    )
                for b in range(DC):  # output dim chunk
                    for j in range(M // NB):  # row half
                        ps = proj_ps.tile([P, NB], FP32, tag="proj_ps")
                        for a in range(DC):  # contraction chunk
                            nc.tensor.matmul(
                                ps[:],
                                lhsT=z_sb[:, a * EMBED + b * P : a * EMBED + (b + 1) * P],
                                rhs=xt_sb[:, a * M + j * NB : a * M + (j + 1) * NB],
                                start=(a == 0),
                                stop=(a == DC - 1),
                            )
                        sl = slice(b * M + j * NB, b * M + (j + 1) * NB)
                        nc.vector.tensor_copy(out=qt_hi[:, sl], in_=ps[:])
                        nc.vector.tensor_tensor(
                            out=qt_lo[:, sl], in0=ps[:], in1=qt_hi[:, sl], op=SUB
                        )

            qh_v = qt_hi[:].rearrange("p (b m) -> p b m", b=DC)  # [P, DC, M]
            ql_v = qt_lo[:].rearrange("p (b m) -> p b m", b=DC)

            # ---- Phase B: streaming attention over the 16 key blocks
            with (
                tc.tile_pool(name="kv", bufs=3) as kvp,
                tc.tile_pool(name="pb", bufs=2) as pbp,
                tc.tile_pool(name="ps_s", bufs=4, space="PSUM") as ps_sp,
                tc.tile_pool(name="ps_u", bufs=2, space="PSUM") as ps_up,
                tc.tile_pool(name="ps_o", bufs=2, space="PSUM") as ps_op,
                tc.tile_pool(name="fin", bufs=2) as fin,
            ):
                scol = fin.tile([P, MB], FP32)
                rtot = fin.tile([P, MB], FP32)
                ones2_v = ones8[:].rearrange("p (s q) -> p s q", s=2)
                # process key blocks in PAIRS: PV and sums accumulate both
                # blocks of a pair inside one PSUM group, halving the DVE
                # accumulate traffic into out_acc/sums_acc.
                for np_ in range(NNB // 2):
                    pts, pt8s, vts = [], [], []
                    for blk in range(2):
                        nb = 2 * np_ + blk
                        vtile = kvp.tile([P, VC * EMBED], FP16, tag=f"vt{blk}")
                        for c in range(VC):
                            nc.sync.dma_start(
                                out=vtile[:, c * EMBED : (c + 1) * EMBED],
                                in_=xv_r[nb * VC + c],
                            )
                        ktile = kvp.tile([P, DC * NB], FP8, tag=f"kt{blk}")
                        for b in range(DC):
                            nc.sync.dma_start(
                                out=ktile[:, b * NB : (b + 1) * NB],
                                in_=xt8_r[b, :, nb * NB : (nb + 1) * NB],
                            )
                        kt_v = ktile[:].rearrange("p (b n) -> p b n", b=DC)

                        pt_sb = pbp.tile([P, VC * M], FP16, tag=f"pt{blk}")
                        pt8_sb = pbp.tile([P, VC * M], FP8, tag=f"pt8_{blk}")
                        for h in range(M // NB):  # query column half
                            for c in range(VC):  # key chunk within block
                                ps_s = ps_sp.tile([P, NB], FP32, tag="ps_s")
                                # fp8 DoubleRow: 2 contraction tiles per
                                # instruction; hi over all DC tiles, lo over
                                # LOC tiles whose position alternates per key
                                # chunk (error decorrelation across key sets).
                                lo0 = 0 if c % 2 == 0 else DC - LOC
                                for qi, q_v, b0, nbb in (
                                    (0, qh_v, 0, DC // 2),
                                    (1, ql_v, lo0, LOC // 2),
                                ):
                                    for bb in range(nbb):
                                        s0 = b0 + 2 * bb
                                        nc.tensor.matmul(
                                            ps_s[:],
                                            lhsT=kt_v[
                                                :,
                                                s0 : s0 + 2,
                                                c * P : (c + 1) * P,
                                            ],
                                            rhs=q_v[
                                                :,
                                                s0 : s0 + 2,
                                                h * NB : (h + 1) * NB,
                                            ],
                                            start=(qi == 0 and bb == 0),
                                            stop=(qi == 1 and bb == LOC // 2 - 1),
                                            perf_mode=DROW,
                                        )
                                csl = slice(c * M + h * NB, c * M + (h + 1) * NB)
                                nc.scalar.activation(
                                    out=pt_sb[:, csl], in_=ps_s[:],
                                    func=EXP, scale=SCALE,
                                )
                                # fp8 copy of P^T feeds the DoubleRow row-sums
                                # (on DVE so ScalarE stays off the critical path)
                                nc.vector.tensor_copy(
                                    out=pt8_sb[:, csl], in_=pt_sb[:, csl]
                                )
                        pts.append(pt_sb)
                        pt8s.append(pt8_sb)
                        vts.append(vtile)

                    # partition-dim softmax sums: fp8 DoubleRow ones-matmul
                    # over both blocks of the pair (2 key chunks/instruction)
                    for h in range(M // NB):
                        ps_sum = ps_up.tile([P, NB], FP32, tag="ps_sum")
                        for blk in range(2):
                            p8_v = pt8s[blk][:].rearrange("p (c m) -> p c m", c=VC)
                            for cc in range(VC // 2):
                                nc.tensor.matmul(
                                    ps_sum[:],
                                    lhsT=ones2_v,
                                    rhs=p8_v[
                                        :, 2 * cc : 2 * cc + 2, h * NB : (h + 1) * NB
                                    ],
                                    start=(blk == 0 and cc == 0),
                                    stop=(blk == 1 and cc == VC // 2 - 1),
                                    perf_mode=DROW,
                                )
                        dsts = sums_acc[:, h * NB : (h + 1) * NB]
                        if np_ == 0:
                            nc.vector.tensor_copy(out=dsts, in_=ps_sum[:])
                        else:
                            nc.vector.tensor_tensor(
                                out=dsts, in0=dsts, in1=ps_sum[:], op=ADD
                            )
                    if np_ == NNB // 2 - 1:
                        # softmax denominators are final: reciprocal per-query
                        # scalars now, so the divides pipeline with the last
                        # pair's PV below.
                        for mb in range(MB):
                            ps_f = ps_up.tile([P, NB], FP32, tag="ps_sum")
                            nc.tensor.transpose(
                                out=ps_f[:, 0:P],
                                in_=sums_acc[:, mb * P : (mb + 1) * P],
                                identity=ident[:],
                            )
                            nc.vector.tensor_copy(
                                out=scol[:, mb : mb + 1], in_=ps_f[:, 0:1]
                            )
                        nc.vector.reciprocal(out=rtot[:], in_=scol[:])
                    for mb in range(MB):
                        for h in range(EMBED // NB):
                            ps_o = ps_op.tile([P, NB], FP32, tag="ps_o")
                            for blk in range(2):
                                for t in range(VC):
                                    nc.tensor.matmul(
                                        ps_o[:],
                                        lhsT=pts[blk][:, t * M + mb * P : t * M + (mb + 1) * P],
                                        rhs=vts[blk][:, t * EMBED + h * NB : t * EMBED + (h + 1) * NB],
                                        start=(blk == 0 and t == 0),
                                        stop=(blk == 1 and t == VC - 1),
                                    )
                            dst = out_acc[:, mb * EMBED + h * NB : mb * EMBED + (h + 1) * NB]
                            if np_ == 0:
                                nc.vector.tensor_copy(out=dst, in_=ps_o[:])
                            else:
                                nc.vector.tensor_tensor(
                                    out=dst, in0=dst, in1=ps_o[:], op=ADD
                                )
                        if np_ == NNB // 2 - 1:
                            # this query block is complete: divide by the
                            # softmax sum and stream it out immediately
                            outf = fin.tile([P, EMBED], BF16, tag="outf")
                            nc.vector.tensor_scalar_mul(
                                outf[:],
                                out_acc[:, mb * EMBED : (mb + 1) * EMBED],
                                rtot[:, mb : mb + 1],
                            )
                            nc.sync.dma_start(out=out_r[mb], in_=outf[:])

    nc.compile()
    return nc


_NC = None


def _get_nc():
    global _NC
    if _NC is None:
        _NC = _build()
    return _NC


def _run(x, rotation_params, entangle_params, **spmd_kwargs):
    x = np.ascontiguousarray(np.asarray(x, dtype=np.float32))
    wq = np.asarray(rotation_params, dtype=np.float32).reshape(EMBED, EMBED)
    wk = np.asarray(entangle_params, dtype=np.float32).reshape(EMBED, EMBED)
    import ml_dtypes

    # offline weight folding: Z = 8 * Wq Wk^T (the 8x keeps the device-side
    # fp8 residual of q~ = x @ 8Z in e4m3's normal range; undone in the exp)
    z8 = (8.0 * (wq @ wk.T)).astype(np.float16)
    xt = np.ascontiguousarray(x.T)
    xt16 = xt.astype(np.float16)
    xt8 = xt.astype(ml_dtypes.float8_e4m3)
    x16 = x.astype(np.float16)
    in_maps = [
        {
            "xt_shard": np.ascontiguousarray(xt16[:, i * M : (i + 1) * M]),
            "xt8_full": xt8,
            "x16_full": x16,
            "z": z8,
        }
        for i in range(NCORES)
    ]
    res = bass_utils.run_bass_kernel_spmd(
        _get_nc(), in_maps, core_ids=list(range(NCORES)), **spmd_kwargs
    )
    out = np.concatenate(
        [res.results[i]["out"].astype(np.float32) for i in range(NCORES)], axis=0
    )
    return out, res


def kernel(x, rotation_params, entangle_params):
    out, _ = _run(x, rotation_params, entangle_params)
    return out
